# revision 1
# baseline (speedup 1.0000x reference)
"""AttentiveMLP GNN message-passing kernel for 8 Trainium2 NeuronCores.

Sharding: edges are partitioned BY DESTINATION NODE (each core owns N/8 nodes
plus all their incoming edges) so no cross-core collectives are needed. Nodes
are grouped on the host into exact-degree classes (a pure layout/permutation
choice); within a class every node has exactly d edges, so segment softmax and
the attention-weighted aggregation are static dense reductions over [nodes, d]
tiles. Aggregation uses linearity: ctx = (sum alpha_e ef_e) @ W_et + b_et.

Device layout: class arrays are staged host-side in the exact SBUF layout
[128 partitions, T*d] (node row = t*128 + p), so each class is one contiguous
per-partition DMA. Edge/node features travel as bf16 to halve DMA traffic.
"""
import os
import numpy as np
import ml_dtypes
from contextlib import ExitStack

import concourse.bass as bass
import concourse.bacc as bacc
import concourse.tile as tile
import concourse.mybir as mybir
from concourse.bass_utils import run_bass_kernel_spmd

N_NODES = 100000
N_EDGES = 1600000
EF = 16
HID = 32
NF = 128
NCORES = 8
CHUNK = 512

f32 = mybir.dt.float32
bf16 = mybir.dt.bfloat16
BF = ml_dtypes.bfloat16


def _build_plan(dst):
    deg = np.bincount(dst, minlength=N_NODES)
    order = np.argsort(deg, kind="stable")
    sdeg = deg[order]
    uniq, starts, counts = np.unique(sdeg, return_index=True, return_counts=True)
    ncls = len(uniq)
    rank = np.arange(N_NODES) - np.repeat(starts, counts)
    dev = rank % NCORES
    row_in_class = rank // NCORES
    n_pad = (counts + NCORES - 1) // NCORES

    cls_ids = list(range(ncls))
    if uniq[0] == 0:
        cls_ids = cls_ids[1:] + [cls_ids[0]]
    offs_arr = np.zeros(ncls, dtype=np.int64)
    acc = 0
    for ci in cls_ids:
        offs_arr[ci] = acc
        acc += n_pad[ci]
    R = int(acc)
    R_pad = ((R + CHUNK - 1) // CHUNK) * CHUNK

    cls_of_pos = np.repeat(np.arange(ncls), counts)
    lrow = offs_arr[cls_of_pos] + row_in_class

    node_dev = np.empty(N_NODES, dtype=np.int64)
    node_lrow = np.empty(N_NODES, dtype=np.int64)
    node_dev[order] = dev
    node_lrow[order] = lrow

    classes = [(int(uniq[ci]), int(n_pad[ci]), int(offs_arr[ci])) for ci in cls_ids]
    deg0_rows = classes[-1][1] if classes and classes[-1][0] == 0 else 0
    kclasses = [c for c in classes if c[0] > 0]
    zero_tail_start = R - deg0_rows

    return dict(
        uniq=uniq, counts=counts, node_dev=node_dev, node_lrow=node_lrow,
        R=R, R_pad=R_pad, kclasses=kclasses, zero_tail_start=zero_tail_start,
    )


def _shard_inputs(inputs, plan):
    lg = np.ascontiguousarray(
        np.asarray(inputs["edge_logits"], dtype=np.float32).reshape(-1))
    ef = np.ascontiguousarray(np.asarray(inputs["edge_feats"], dtype=np.float32))
    nf = np.asarray(inputs["node_feats"], dtype=np.float32)
    dst = np.asarray(inputs["dst"])
    W_et = np.asarray(inputs["W_et"], dtype=np.float32)
    b_et = np.asarray(inputs["b_et"], dtype=np.float32)
    W1 = np.asarray(inputs["W1"], dtype=np.float32)
    b1 = np.asarray(inputs["b1"], dtype=np.float32)
    W2 = np.asarray(inputs["W2"], dtype=np.float32)
    b2 = np.asarray(inputs["b2"], dtype=np.float32)

    node_dev, node_lrow = plan["node_dev"], plan["node_lrow"]
    R, R_pad = plan["R"], plan["R_pad"]
    kclasses = plan["kclasses"]
    uniq, counts = plan["uniq"], plan["counts"]

    ekey = node_dev[dst] * R + node_lrow[dst]
    eorder = np.argsort(ekey, kind="stable")
    cnt_of_deg = {int(d): int(c) for d, c in zip(uniq, counts)}

    def n_real(d, dv):
        c = cnt_of_deg[d]
        return c // NCORES + (1 if dv < (c % NCORES) else 0)

    in_maps = [dict() for _ in range(NCORES)]
    pos = 0
    for dv in range(NCORES):
        for idx, (d, npad, off) in enumerate(kclasses):
            T = (npad + 127) // 128
            nr = n_real(d, dv)
            ne = nr * d
            block = eorder[pos:pos + ne]
            pos += ne
            lg_c = np.zeros((T * 128, d), dtype=np.float32)
            ef_c = np.zeros((T * 128, d * EF), dtype=BF)
            lg_c[:nr] = lg[block].reshape(nr, d)
            ef_c[:nr] = ef[block].reshape(nr, d * EF).astype(BF)
            # row r = t*128+p  ->  host layout [128, T*d]
            in_maps[dv][f"lg{idx}"] = np.ascontiguousarray(
                lg_c.reshape(T, 128, d).transpose(1, 0, 2).reshape(128, T * d))
            in_maps[dv][f"ef{idx}"] = np.ascontiguousarray(
                ef_c.reshape(T, 128, d * EF).transpose(1, 0, 2).reshape(128, T * d * EF))
    assert pos == N_EDGES

    for dv in range(NCORES):
        sel = node_dev == dv
        nid = np.nonzero(sel)[0]
        lr = node_lrow[sel]
        nf_dev = np.zeros((R_pad, NF), dtype=np.float32)
        nf_dev[lr] = nf[nid]
        in_maps[dv]["nfT"] = np.ascontiguousarray(nf_dev.T).astype(BF)

    wet4 = np.zeros((128, 128), dtype=BF)
    bet4 = np.zeros((128, 1), dtype=np.float32)
    for g in range(4):
        wet4[32 * g:32 * g + EF, 32 * g:32 * g + HID] = W_et.astype(BF)
        bet4[32 * g:32 * g + HID, 0] = b_et
    consts = {
        "wet4": wet4,
        "bet4": bet4,
        "w1c": np.ascontiguousarray(np.tile(W1[:HID], (4, 1))).astype(BF),
        "w1n": np.ascontiguousarray(W1[HID:]).astype(BF),
        "b1": b1.reshape(NF, 1).astype(np.float32),
        "w2": W2.astype(BF),
        "b2": b2.reshape(NF, 1).astype(np.float32),
        "ident": np.eye(128, dtype=BF),
    }
    for dv in range(NCORES):
        in_maps[dv].update({k: v.copy() for k, v in consts.items()})
    return in_maps


def _unshard(results, plan):
    node_dev, node_lrow = plan["node_dev"], plan["node_lrow"]
    out = np.empty((N_NODES, NF), dtype=np.float32)
    for dv in range(NCORES):
        sel = node_dev == dv
        nid = np.nonzero(sel)[0]
        lr = node_lrow[sel]
        out_dev = results[dv]["outT"].T
        out[nid] = out_dev[lr]
    return out


def _build_kernel(plan):
    kclasses = plan["kclasses"]
    R_pad = plan["R_pad"]
    zts = plan["zero_tail_start"]
    n_chunks = R_pad // CHUNK
    n_blocks = (n_chunks + 3) // 4
    agg_cols = n_blocks * CHUNK

    nc = bacc.Bacc("TRN2", target_bir_lowering=False, debug=False,
                   num_devices=NCORES)

    lg_d, ef_d = [], []
    for idx, (d, npad, off) in enumerate(kclasses):
        T = (npad + 127) // 128
        lg_d.append(nc.dram_tensor(f"lg{idx}", [128, T * d], f32, kind="ExternalInput"))
        ef_d.append(nc.dram_tensor(f"ef{idx}", [128, T * d * EF], bf16, kind="ExternalInput"))
    nfT_d = nc.dram_tensor("nfT", [NF, R_pad], bf16, kind="ExternalInput")
    wet4_d = nc.dram_tensor("wet4", [128, 128], bf16, kind="ExternalInput")
    bet4_d = nc.dram_tensor("bet4", [128, 1], f32, kind="ExternalInput")
    w1c_d = nc.dram_tensor("w1c", [NF, NF], bf16, kind="ExternalInput")
    w1n_d = nc.dram_tensor("w1n", [NF, NF], bf16, kind="ExternalInput")
    b1_d = nc.dram_tensor("b1", [NF, 1], f32, kind="ExternalInput")
    w2_d = nc.dram_tensor("w2", [NF, NF], bf16, kind="ExternalInput")
    b2_d = nc.dram_tensor("b2", [NF, 1], f32, kind="ExternalInput")
    ident_d = nc.dram_tensor("ident", [128, 128], bf16, kind="ExternalInput")
    out_d = nc.dram_tensor("outT", [NF, R_pad], f32, kind="ExternalOutput")

    with tile.TileContext(nc) as tc, ExitStack() as ctx:
        const_pool = ctx.enter_context(tc.tile_pool(name="const", bufs=1))
        agg_pool = ctx.enter_context(tc.tile_pool(name="agg", bufs=1))
        cls_pool = ctx.enter_context(tc.tile_pool(name="cls", bufs=3))
        work_pool = ctx.enter_context(tc.tile_pool(name="work", bufs=3))
        small_pool = ctx.enter_context(tc.tile_pool(name="small", bufs=4))
        mlp_pool = ctx.enter_context(tc.tile_pool(name="mlp", bufs=3))
        ctx_pool = ctx.enter_context(tc.tile_pool(name="ctxs", bufs=2))
        tr_psum = ctx.enter_context(tc.tile_pool(name="trp", bufs=2, space="PSUM"))
        ctx_psum = ctx.enter_context(tc.tile_pool(name="ctxp", bufs=2, space="PSUM"))
        mlp1_psum = ctx.enter_context(tc.tile_pool(name="m1p", bufs=2, space="PSUM"))
        mlp2_psum = ctx.enter_context(tc.tile_pool(name="m2p", bufs=2, space="PSUM"))

        def load_const(name, dram, shape, dtype=f32):
            t = const_pool.tile(shape, dtype, name=name)
            nc.sync.dma_start(t[:], dram.ap())
            return t

        identb = load_const("identc", ident_d, [128, 128], bf16)
        wet4 = load_const("wet4c", wet4_d, [128, 128], bf16)
        bet4 = load_const("bet4c", bet4_d, [128, 1])
        w1c = load_const("w1cc", w1c_d, [NF, NF], bf16)
        w1n = load_const("w1nc", w1n_d, [NF, NF], bf16)
        w2 = load_const("w2c", w2_d, [NF, NF], bf16)
        b1 = load_const("b1c", b1_d, [NF, 1])
        b2 = load_const("b2c", b2_d, [NF, 1])
        zeros = const_pool.tile([128, CHUNK], f32, name="zeros")
        nc.vector.memset(zeros[:], 0.0)

        aggT = agg_pool.tile([128, agg_cols], bf16, name="aggT")
        nc.gpsimd.memset(aggT[:], 0.0)

        # staging flush: copy stage[0:16, 0:length] -> aggT at global row gr0,
        # splitting at 512-chunk boundaries; alternate ACT/DVE.
        flush_ctr = [0]

        def flush(stage, gr0, length):
            s = 0
            while s < length:
                r = gr0 + s
                g = (r // CHUNK) % 4
                b = r // (4 * CHUNK)
                c0 = b * CHUNK + (r % CHUNK)
                seg = min(length - s, CHUNK - (r % CHUNK))
                dstv = aggT[32 * g:32 * g + EF, c0:c0 + seg]
                srcv = stage[0:EF, s:s + seg]
                if flush_ctr[0] % 2 == 0:
                    nc.scalar.copy(dstv, srcv)
                else:
                    nc.vector.tensor_copy(dstv, srcv)
                flush_ctr[0] += 1
                s += seg

        # ---------------- Phase A ----------------
        for idx, (d, npad, off) in enumerate(kclasses):
            T = (npad + 127) // 128
            lgt = cls_pool.tile([128, T * d], f32, tag="lg", name=f"lgt{idx}")
            eft = cls_pool.tile([128, T * d * EF], bf16, tag="ef", name=f"eft{idx}")
            nc.gpsimd.dma_start(lgt[:], lg_d[idx].ap())
            nc.tensor.dma_start(eft[:], ef_d[idx].ap())

            lg3 = lgt[:].rearrange("p (t d) -> p t d", t=T)
            nm = small_pool.tile([128, T], f32, tag="nm", name=f"nm{idx}")
            nc.vector.tensor_reduce(nm[:], lg3, mybir.AxisListType.X,
                                    mybir.AluOpType.max, negate=True)
            lgc = work_pool.tile([128, T * d], f32, tag="lgc", name=f"lgc{idx}")
            nm_b = nm[:].unsqueeze(2).broadcast_to([128, T, d])
            lgc3 = lgc[:].rearrange("p (t d) -> p t d", t=T)
            nc.vector.tensor_tensor(lgc3, lg3, nm_b, mybir.AluOpType.add)

            x = work_pool.tile([128, T * d], bf16, tag="x", name=f"x{idx}")
            nc.scalar.activation(x[:], lgc[:], mybir.ActivationFunctionType.Exp)
            x3 = x[:].rearrange("p (t d) -> p t d", t=T)
            den = small_pool.tile([128, T], f32, tag="den", name=f"den{idx}")
            nc.vector.tensor_reduce(den[:], x3, mybir.AxisListType.X,
                                    mybir.AluOpType.add)
            rd = small_pool.tile([128, T], f32, tag="rd", name=f"rd{idx}")
            nc.vector.reciprocal(rd[:], den[:])
            xs = work_pool.tile([128, T * d], bf16, tag="xs", name=f"xs{idx}")
            xs3 = xs[:].rearrange("p (t d) -> p t d", t=T)
            rd_b = rd[:].unsqueeze(2).broadcast_to([128, T, d])
            nc.vector.tensor_tensor(xs3, x3, rd_b, mybir.AluOpType.mult)

            prod = work_pool.tile([128, T * d * EF], bf16, tag="prod", name=f"prod{idx}")
            ef4 = eft[:].rearrange("p (t d f) -> p t d f", t=T, d=d)
            xs4 = xs3.unsqueeze(3).broadcast_to([128, T, d, EF])
            prod4 = prod[:].rearrange("p (t d f) -> p t d f", t=T, d=d)
            nc.vector.tensor_tensor(prod4, ef4, xs4, mybir.AluOpType.mult)

            a16 = work_pool.tile([128, T * EF], f32, tag="a16", name=f"a16{idx}")
            a16_3 = a16[:].rearrange("p (t f) -> p t f", t=T)
            prod_r = prod[:].rearrange("p (t d f) -> p t f d", t=T, d=d)
            nc.vector.tensor_reduce(a16_3, prod_r, mybir.AxisListType.X,
                                    mybir.AluOpType.add)
            a16b = work_pool.tile([128, T * EF], bf16, tag="a16b", name=f"a16b{idx}")
            nc.scalar.copy(a16b[:], a16[:])

            # transposes into class-local 512-col staging windows
            stage = None
            st_base = 0      # class-local col of stage[:, 0]
            st_len = 0
            for t in range(T):
                p = min(128, npad - t * 128)
                if p <= 0:
                    break
                if stage is None:
                    stage = tr_psum.tile([EF, CHUNK], bf16, tag="st", name=f"st{idx}_{t}")
                    st_base = t * 128
                    st_len = 0
                q = t * 128 - st_base
                nc.tensor.matmul(stage[0:EF, q:q + p],
                                 a16b[0:p, t * EF:(t + 1) * EF],
                                 identb[0:p, 0:p], is_transpose=True)
                st_len = q + p
                if st_len == CHUNK:
                    flush(stage, off + st_base, st_len)
                    stage = None
            if stage is not None and st_len > 0:
                flush(stage, off + st_base, st_len)

        # ---------------- Phases B+C ----------------
        for b in range(n_blocks):
            ctx4 = ctx_psum.tile([128, CHUNK], f32, tag="ctx4", name=f"ctx4_{b}")
            nc.tensor.matmul(ctx4[:], wet4[:], aggT[:, b * CHUNK:(b + 1) * CHUNK])
            s1 = mlp_pool.tile([128, CHUNK], f32, tag="s1", name=f"s1_{b}")
            nc.scalar.activation(s1[:], ctx4[:], mybir.ActivationFunctionType.Relu,
                                 bias=bet4[:], scale=1.0)
            s2 = mlp_pool.tile([128, CHUNK], f32, tag="s2", name=f"s2_{b}")
            nc.scalar.activation(s2[:], ctx4[:], mybir.ActivationFunctionType.Exp,
                                 bias=bet4[:], scale=1.0)
            zm = mlp_pool.tile([128, CHUNK], f32, tag="zm", name=f"zm_{b}")
            nc.vector.scalar_tensor_tensor(zm[:], s2[:], -1.0, zeros[:],
                                           mybir.AluOpType.add, mybir.AluOpType.min)
            cb = ctx_pool.tile([128, CHUNK], bf16, tag="cb", name=f"cb_{b}")
            nc.vector.tensor_add(cb[:], s1[:], zm[:])
            for g in range(4):
                n0 = (4 * b + g) * CHUNK
                z0 = max(zts, n0)
                if z0 < n0 + CHUNK:
                    nc.vector.memset(cb[32 * g:32 * (g + 1), z0 - n0:CHUNK], 0.0)

            for g in range(4):
                j = 4 * b + g
                if j >= n_chunks:
                    break
                nfb = mlp_pool.tile([NF, CHUNK], bf16, tag="nfb", name=f"nfb{j}")
                nc.sync.dma_start(nfb[:], nfT_d.ap()[:, j * CHUNK:(j + 1) * CHUNK])
                ps1 = mlp1_psum.tile([NF, CHUNK], f32, tag="ps1", name=f"ps1_{j}")
                if g < 3:
                    cbg = cb[32 * g:32 * (g + 1), :]
                    w1cg = w1c[32 * g:32 * (g + 1), :]
                else:
                    cb3 = mlp_pool.tile([HID, CHUNK], bf16, tag="cb3", name=f"cb3_{j}")
                    nc.vector.tensor_copy(cb3[:], cb[96:128, :])
                    cbg = cb3[:]
                    w1cg = w1c[0:HID, :]
                nc.tensor.matmul(ps1[:], w1cg, cbg, start=True, stop=False)
                nc.tensor.matmul(ps1[:], w1n[:], nfb[:], start=False, stop=True)
                h = mlp_pool.tile([NF, CHUNK], bf16, tag="h", name=f"h{j}")
                nc.scalar.activation(h[:], ps1[:], mybir.ActivationFunctionType.Relu,
                                     bias=b1[:], scale=1.0)
                ps2 = mlp2_psum.tile([NF, CHUNK], f32, tag="ps2", name=f"ps2_{j}")
                nc.tensor.matmul(ps2[:], w2[:], h[:])
                o = mlp_pool.tile([NF, CHUNK], f32, tag="o", name=f"o{j}")
                nc.scalar.activation(o[:], ps2[:], mybir.ActivationFunctionType.Relu,
                                     bias=b2[:], scale=1.0)
                nc.sync.dma_start(out_d.ap()[:, j * CHUNK:(j + 1) * CHUNK], o[:])

    nc.compile()
    return nc


def kernel(**inputs):
    dst = np.asarray(inputs["dst"])
    plan = _build_plan(dst)
    in_maps = _shard_inputs(inputs, plan)
    nc = _build_kernel(plan)
    trace = bool(int(os.environ.get("GNN_PROFILE", "0")))
    if trace:
        try:
            _install_ntff_hook()
        except Exception:
            pass
    res = run_bass_kernel_spmd(nc, in_maps, core_ids=list(range(NCORES)),
                               trace=trace)
    kernel.last_results = res
    return _unshard(res.results, plan)


def _install_ntff_hook():
    """Recreate antenv.axon_hooks (absent in this image) so
    run_bass_kernel_spmd(trace=True) can NTFF-profile via libaxon_pjrt.so."""
    import contextlib, ctypes, sys, types
    if 'antenv.axon_hooks' in sys.modules:
        return
    lib = ctypes.CDLL('/opt/axon/libaxon_pjrt.so')
    lib.axon_start_nrt_profile.argtypes = [ctypes.POINTER(ctypes.c_int64), ctypes.c_size_t]
    lib.axon_start_nrt_profile.restype = ctypes.c_int64
    lib.axon_stop_nrt_profile.argtypes = [ctypes.c_char_p]
    lib.axon_stop_nrt_profile.restype = ctypes.c_int64

    @contextlib.contextmanager
    def _hook(output_dir, device_ids):
        import jax
        jax.devices()
        if device_ids:
            ids = (ctypes.c_int64 * len(device_ids))(*device_ids)
            rc = lib.axon_start_nrt_profile(ids, len(device_ids))
        else:
            rc = lib.axon_start_nrt_profile(None, 0)
        if rc != 0:
            raise RuntimeError(f"axon_start_nrt_profile rc={rc}")
        try:
            yield
        finally:
            n = lib.axon_stop_nrt_profile(str(output_dir).encode())
            print(f"ntff profile: {n} file(s) written to {output_dir}", file=sys.stderr)

    mod = types.ModuleType('antenv.axon_hooks')
    mod.get_axon_ntff_profile_hook = lambda: _hook
    mod.set_axon_ntff_profile_hook = lambda h: None
    import antenv
    antenv.axon_hooks = mod
    sys.modules['antenv.axon_hooks'] = mod
